# revision 1
# baseline (speedup 1.0000x reference)
"""MoE network TRN2 kernel: 8-way data-parallel over the batch.

Per core: 512 tokens. All activations kept in transposed [feature, token]
layout so BatchNorm reduces along the free dim. Expert matmuls run in
float32r (full PE rate); gating logits in float32 (exact top-2 routing).
BatchNorm statistics are the only cross-core communication (tiny AllReduce).
"""
import os
import sys

import numpy as np

sys.path.insert(0, "/opt/trn_rl_repo")

B, DIN, DHID, DH2, E = 4096, 1024, 2048, 1024, 8
NCORES = 8
BL = B // NCORES            # 512 tokens per core
IC1 = DIN // 128            # 8  input chunks, layer 1
JC1 = DHID // 128           # 16 output chunks, layer 1
IC2 = DHID // 128           # 16
JC2 = DH2 // 128            # 8
TC = BL // 128              # 4  token chunks per core
EPS = 1e-5

_CACHE = {}


def _round_fp32r(x):
    """fp32r = fp32 rounded to 11 mantissa bits, round-to-nearest-even
    (verified bit-exact against the DVE fp32->fp32r cast on hardware)."""
    b = np.ascontiguousarray(x, np.float32).view(np.uint32).astype(np.uint64)
    half = np.uint64(1 << 11)
    one = np.uint64(1)
    lsb = (b >> np.uint64(12)) & one
    b = (b + half - one + lsb) & ~np.uint64((1 << 12) - 1)
    return (b & np.uint64(0xFFFFFFFF)).astype(np.uint32).view(np.float32)


def _build(reps=1, py_unroll=False):
    import concourse.bass_isa as bass_isa
    import concourse.mybir as mybir
    import concourse.tile as tile
    from concourse import bacc
    from contextlib import nullcontext

    f32 = mybir.dt.float32
    f32r = mybir.dt.float32r
    AF = mybir.ActivationFunctionType
    OP = mybir.AluOpType
    RG = [list(range(NCORES))]

    nc = bacc.Bacc(None, target_bir_lowering=False, num_devices=NCORES)

    xt = nc.dram_tensor("xt", [DIN, BL], f32, kind="ExternalInput")
    xfull = nc.dram_tensor("xfull", [DIN, B], f32, kind="ExternalInput")
    w1 = nc.dram_tensor("w1", [E, IC1, 128, DHID], f32r, kind="ExternalInput")
    w2 = nc.dram_tensor("w2", [E, IC2, 128, DH2], f32r, kind="ExternalInput")
    b1 = nc.dram_tensor("b1", [JC1, E, 128], f32, kind="ExternalInput")
    b2 = nc.dram_tensor("b2", [JC2, E, 128], f32, kind="ExternalInput")
    g1w = nc.dram_tensor("g1w", [IC1, 128, E], f32, kind="ExternalInput")
    g2w = nc.dram_tensor("g2w", [IC2, 128, E], f32, kind="ExternalInput")
    g1b = nc.dram_tensor("g1b", [E, 1], f32, kind="ExternalInput")
    g2b = nc.dram_tensor("g2b", [E, 1], f32, kind="ExternalInput")
    bn1g = nc.dram_tensor("bn1g", [IC1, 128], f32, kind="ExternalInput")
    bn1b = nc.dram_tensor("bn1b", [IC1, 128], f32, kind="ExternalInput")
    bn2g = nc.dram_tensor("bn2g", [IC2, 128], f32, kind="ExternalInput")
    bn2b = nc.dram_tensor("bn2b", [IC2, 128], f32, kind="ExternalInput")
    ow = nc.dram_tensor("ow", [JC2, 128], f32, kind="ExternalInput")
    ob = nc.dram_tensor("ob", [1, 1], f32, kind="ExternalInput")
    out = nc.dram_tensor("out", [BL, 1], f32, kind="ExternalOutput")

    with tile.TileContext(nc) as tc:
        with tc.tile_pool(name="const", bufs=1) as const, \
             tc.tile_pool(name="res", bufs=1) as res, \
             tc.tile_pool(name="wpool", bufs=12) as wpool, \
             tc.tile_pool(name="hpool", bufs=4) as hpool, \
             tc.tile_pool(name="small", bufs=1) as small, \
             tc.tile_pool(name="gsc", bufs=10) as gsc, \
             tc.tile_pool(name="dram", bufs=1, space="DRAM") as dram:

            # ------- small parameter loads
            bn1g_t = const.tile([128, IC1], f32)
            bn1b_t = const.tile([128, IC1], f32)
            bn2g_t = const.tile([128, IC2], f32)
            bn2b_t = const.tile([128, IC2], f32)
            nc.sync.dma_start(out=bn1g_t[:], in_=bn1g.rearrange("c p -> p c"))
            nc.sync.dma_start(out=bn1b_t[:], in_=bn1b.rearrange("c p -> p c"))
            nc.sync.dma_start(out=bn2g_t[:], in_=bn2g.rearrange("c p -> p c"))
            nc.sync.dma_start(out=bn2b_t[:], in_=bn2b.rearrange("c p -> p c"))
            g1w_t = const.tile([128, IC1, E], f32)
            g2w_t = const.tile([128, IC2, E], f32)
            nc.sync.dma_start(out=g1w_t[:], in_=g1w.rearrange("c p e -> p c e"))
            nc.sync.dma_start(out=g2w_t[:], in_=g2w.rearrange("c p e -> p c e"))
            g1b_t = const.tile([E, 1], f32)
            g2b_t = const.tile([E, 1], f32)
            nc.sync.dma_start(out=g1b_t[:], in_=g1b[:])
            nc.sync.dma_start(out=g2b_t[:], in_=g2b[:])
            b1_tf = small.tile([E, JC1, 128], f32, tag="btmp", name="b1_tf")
            nc.sync.dma_start(out=b1_tf[:], in_=b1.rearrange("j e p -> e j p"))
            b1_t = const.tile([E, JC1, 128], f32r)
            nc.vector.tensor_copy(b1_t[:], b1_tf[:])
            b2_tf = small.tile([E, JC2, 128], f32, tag="btmp", name="b2_tf")
            nc.sync.dma_start(out=b2_tf[:], in_=b2.rearrange("j e p -> e j p"))
            b2_t = const.tile([E, JC2, 128], f32r)
            nc.vector.tensor_copy(b2_t[:], b2_tf[:])
            ow_t = const.tile([128, JC2], f32)
            nc.sync.dma_start(out=ow_t[:], in_=ow.rearrange("c p -> p c"))
            ob_t = const.tile([128, 1], f32)
            nc.sync.dma_start(out=ob_t[:], in_=ob[0:1, 0:1].partition_broadcast(128).squeeze(1))

            junk = res.tile([128, 512], f32)

            # ------- batchnorm helpers
            def bn_finish(s1, s2, icn, gamma_t, beta_t, name):
                mu = small.tile([128, icn], f32, name=f"mu_{name}")
                ex2 = small.tile([128, icn], f32, name=f"ex2_{name}")
                nc.vector.tensor_scalar(mu[:], s1[:], 1.0 / B, None, OP.mult)
                nc.vector.tensor_scalar(ex2[:], s2[:], 1.0 / B, None, OP.mult)
                var = small.tile([128, icn], f32, name=f"var_{name}")
                nc.vector.tensor_tensor(out=var[:], in0=mu[:], in1=mu[:], op=OP.mult)
                nc.vector.tensor_tensor(out=var[:], in0=ex2[:], in1=var[:], op=OP.subtract)
                vare = small.tile([128, icn], f32, name=f"vare_{name}")
                nc.vector.tensor_scalar(vare[:], var[:], EPS, None, OP.add)
                sd = small.tile([128, icn], f32, name=f"sd_{name}")
                nc.scalar.activation(sd[:], vare[:], AF.Sqrt)
                rstd = small.tile([128, icn], f32, name=f"rstd_{name}")
                nc.vector.reciprocal(rstd[:], sd[:])
                sv = small.tile([128, icn], f32, name=f"sv_{name}")
                bv = small.tile([128, icn], f32, name=f"bv_{name}")
                nc.vector.tensor_tensor(out=sv[:], in0=rstd[:], in1=gamma_t[:], op=OP.mult)
                nc.vector.tensor_tensor(out=bv[:], in0=mu[:], in1=sv[:], op=OP.mult)
                nc.vector.tensor_tensor(out=bv[:], in0=beta_t[:], in1=bv[:], op=OP.subtract)
                return sv, bv

            # BN1: replicated global stats from the full (all-token) x
            def bn1_stats_replicated():
                TB = 512
                ntb = B // TB
                s1r = small.tile([128, IC1, ntb], f32, name="s1r_bn1")
                s2r = small.tile([128, IC1, ntb], f32, name="s2r_bn1")
                for ic in range(IC1):
                    for tb in range(ntb):
                        xs = hpool.tile([128, TB], f32, tag="xstat", bufs=3,
                                        name=f"xs_{ic}_{tb}")
                        nc.sync.dma_start(
                            out=xs[:],
                            in_=xfull[ic * 128:(ic + 1) * 128,
                                      tb * TB:(tb + 1) * TB])
                        nc.vector.tensor_reduce(
                            s1r[:, ic, tb:tb + 1], xs[:],
                            mybir.AxisListType.X, OP.add)
                        nc.scalar.activation(
                            junk[:], xs[:], AF.Square,
                            accum_out=s2r[:, ic, tb:tb + 1])
                s1 = small.tile([128, IC1], f32, name="s1_bn1")
                s2 = small.tile([128, IC1], f32, name="s2_bn1")
                nc.vector.tensor_reduce(s1[:], s1r[:], mybir.AxisListType.X, OP.add)
                nc.vector.tensor_reduce(s2[:], s2r[:], mybir.AxisListType.X, OP.add)
                return bn_finish(s1, s2, IC1, bn1g_t, bn1b_t, "bn1")

            # BN2: per-half partials, each AllReduced as soon as available
            def bn2_partial(src, jcs, name):
                icn = len(jcs)
                s1 = small.tile([128, icn], f32, name=f"s1_{name}")
                s2 = small.tile([128, icn], f32, name=f"s2_{name}")
                for k, jc in enumerate(jcs):
                    nc.vector.tensor_reduce(
                        s1[:, k:k + 1], src[:, jc, :], mybir.AxisListType.X, OP.add)
                    nc.scalar.activation(
                        junk[:, :BL], src[:, jc, :], AF.Square,
                        accum_out=s2[:, k:k + 1])
                pk = small.tile([128, 2 * icn], f32, name=f"pk_{name}")
                nc.vector.tensor_copy(pk[:, :icn], s1[:])
                nc.vector.tensor_copy(pk[:, icn:], s2[:])
                pl = dram.tile([128, 2 * icn], f32, name=f"bnp_{name}")
                ps = dram.tile([128, 2 * icn], f32, addr_space="Shared",
                               name=f"bns_{name}")
                nc.sync.dma_start(out=pl[:], in_=pk[:])
                if (reps == 1 or py_unroll) and not os.environ.get("KERNEL_NOCC"):
                    nc.gpsimd.collective_compute(
                        "AllReduce", OP.add, replica_groups=RG,
                        ins=[pl[:]], outs=[ps[:]])
                else:  # collectives desync inside For_i; timing-only stub
                    nc.sync.dma_start(out=ps[:], in_=pl[:])
                gl = small.tile([128, 2 * icn], f32, name=f"gl_{name}")
                nc.sync.dma_start(out=gl[:], in_=ps[:])
                return gl

            # ------- gating helper: logitsT [E, BL] -> top-2 masked softmax -> bcast
            def gating(xn, icn, gwt, gbt, gbc, name):
                with tc.tile_pool(name=f"psg_{name}", bufs=1, space="PSUM") as psgp:
                    psg = psgp.tile([E, BL], f32)
                    for ic in range(icn):
                        nc.tensor.matmul(psg[:], lhsT=gwt[:, ic, :], rhs=xn[:, ic, :],
                                         start=(ic == 0), stop=(ic == icn - 1))
                    lg = gsc.tile([E, BL], f32, tag="g", name=f"lg_{name}")
                    nc.vector.tensor_scalar(lg[:], psg[:], gbt[:], None, OP.add)
                m1 = gsc.tile([E, BL], f32, tag="g", name=f"m1_{name}")
                nc.gpsimd.partition_all_reduce(m1[:], lg[:], channels=E,
                                               reduce_op=bass_isa.ReduceOp.max)
                ismax = gsc.tile([E, BL], f32, tag="g", name=f"ismax_{name}")
                nc.vector.tensor_tensor(out=ismax[:], in0=lg[:], in1=m1[:], op=OP.is_equal)
                cnt = gsc.tile([E, BL], f32, tag="g", name=f"cnt_{name}")
                nc.gpsimd.partition_all_reduce(cnt[:], ismax[:], channels=E,
                                               reduce_op=bass_isa.ReduceOp.add)
                tmp = gsc.tile([E, BL], f32, tag="g", name=f"tmp_{name}")
                nc.vector.scalar_tensor_tensor(
                    out=tmp[:], in0=ismax[:], scalar=-1e30, in1=lg[:],
                    op0=OP.mult, op1=OP.add)
                m2 = gsc.tile([E, BL], f32, tag="g", name=f"m2_{name}")
                nc.gpsimd.partition_all_reduce(m2[:], tmp[:], channels=E,
                                               reduce_op=bass_isa.ReduceOp.max)
                c2m = gsc.tile([E, BL], f32, tag="g", name=f"c2m_{name}")
                nc.vector.tensor_scalar(c2m[:], cnt[:], 1.5, None, OP.is_ge)
                dif = gsc.tile([E, BL], f32, tag="g", name=f"dif_{name}")
                nc.vector.tensor_tensor(out=dif[:], in0=m1[:], in1=m2[:], op=OP.subtract)
                nc.vector.tensor_tensor(out=dif[:], in0=dif[:], in1=c2m[:], op=OP.mult)
                v2 = gsc.tile([E, BL], f32, tag="g", name=f"v2_{name}")
                nc.vector.tensor_tensor(out=v2[:], in0=dif[:], in1=m2[:], op=OP.add)
                msk = gsc.tile([E, BL], f32, tag="g", name=f"msk_{name}")
                nc.vector.tensor_tensor(out=msk[:], in0=lg[:], in1=v2[:], op=OP.is_ge)
                d = gsc.tile([E, BL], f32, tag="g", name=f"d_{name}")
                nc.vector.tensor_tensor(out=d[:], in0=lg[:], in1=m1[:], op=OP.subtract)
                exd = gsc.tile([E, BL], f32, tag="g", name=f"exd_{name}")
                nc.scalar.activation(exd[:], d[:], AF.Exp)
                exm = gsc.tile([E, BL], f32, tag="g", name=f"exm_{name}")
                nc.vector.tensor_tensor(out=exm[:], in0=exd[:], in1=msk[:], op=OP.mult)
                den = gsc.tile([E, BL], f32, tag="g", name=f"den_{name}")
                nc.gpsimd.partition_all_reduce(den[:], exm[:], channels=E,
                                               reduce_op=bass_isa.ReduceOp.add)
                rden = gsc.tile([E, BL], f32, tag="g", name=f"rden_{name}")
                nc.vector.reciprocal(rden[:], den[:])
                gat = gsc.tile([E, BL], f32, tag="g", name=f"gat_{name}")
                nc.vector.tensor_tensor(out=gat[:], in0=exm[:], in1=rden[:], op=OP.mult)
                gatr = small.tile([E, BL], f32r, name=f"gatr_{name}")
                nc.vector.tensor_copy(gatr[:], gat[:])
                gd = dram.tile([E, BL], f32, name=f"gd_{name}")
                nc.sync.dma_start(out=gd[:], in_=gat[:])
                for e in range(E):
                    nc.sync.dma_start(
                        out=gbc[:, e, :],
                        in_=gd[e:e + 1, :].partition_broadcast(128).squeeze(1))
                return gat, gatr

            # ------- expert layer helper
            def expert_layer(xn, gat, gbc, wdram, bt, icn, jcn, zdst, relu_out,
                             jh_cb=None):
                n_jh = (jcn + 7) // 8
                with tc.tile_pool(name=f"psm_{len(zdst.shape)}_{icn}", bufs=8,
                                  space="PSUM") as psp:
                    for jh in range(n_jh):
                        njc = min(8, jcn - jh * 8)
                        pss = [psp.tile([128, BL], f32, tag="ps",
                                        name=f"ps_{jh}_{j}") for j in range(njc)]
                        for jc in range(njc):
                            nc.tensor.matmul(pss[jc][:], lhsT=bt[:, jh * 8 + jc, :],
                                             rhs=gat[:], start=True, stop=False)
                        for e in range(E):
                            for ic in range(icn):
                                ws = wpool.tile([128, njc * 128], f32r, tag="ws",
                                                name=f"ws_{jh}_{e}_{ic}")
                                nc.sync.dma_start(
                                    out=ws[:],
                                    in_=wdram[e, ic, :, jh * 1024:jh * 1024 + njc * 128])
                                ht = hpool.tile([128, BL], f32r, tag="ht",
                                                name=f"ht_{jh}_{e}_{ic}")
                                nc.vector.tensor_tensor(
                                    out=ht[:], in0=xn[:, ic, :], in1=gbc[:, e, :],
                                    op=OP.mult)
                                last = (e == E - 1 and ic == icn - 1)
                                for jc in range(njc):
                                    nc.tensor.matmul(
                                        pss[jc][:],
                                        lhsT=ws[:, jc * 128:(jc + 1) * 128],
                                        rhs=ht[:], start=False, stop=last)
                        for jc in range(njc):
                            if relu_out:
                                nc.scalar.activation(zdst[:, jh * 8 + jc, :],
                                                     pss[jc][:], AF.Relu)
                            else:
                                nc.vector.tensor_copy(zdst[:, jh * 8 + jc, :],
                                                      pss[jc][:])
                        if jh_cb is not None:
                            jh_cb(jh, [jh * 8 + j for j in range(njc)])

            def emit_forward():
                # =================== forward pass ===================
                # x load + BN1 stats
                xtf = res.tile([128, IC1, BL], f32, tag="bigA")
                for ic in range(IC1):
                    nc.sync.dma_start(out=xtf[:, ic, :], in_=xt[ic * 128:(ic + 1) * 128, :])
                sv1, bv1 = bn1_stats_replicated()

                # normalize (fp32, exact for gating)
                xnf = res.tile([128, IC1, BL], f32, tag="bigB")
                for ic in range(IC1):
                    nc.vector.tensor_scalar(xnf[:, ic, :], xtf[:, ic, :],
                                            sv1[:, ic:ic + 1], bv1[:, ic:ic + 1],
                                            OP.mult, OP.add)

                g1bc = res.tile([128, E, BL], f32, tag="gbc")
                gat1, gat1r = gating(xnf, IC1, g1w_t, g1b_t, g1bc, "g1")

                z1T = res.tile([128, JC1, BL], f32)
                bn2_gls = {}

                def bn2_cb(jh, jcs):
                    bn2_gls[jh] = bn2_partial(z1T, jcs, f"bn2h{jh}")

                expert_layer(xnf, gat1r, g1bc, w1, b1_t, IC1, JC1, z1T,
                             relu_out=False, jh_cb=bn2_cb)

                # BN2 + ReLU: combine the two halves' global partials
                glA, glB = bn2_gls[0], bn2_gls[1]
                s1c = small.tile([128, JC1], f32, name="s1_bn2")
                s2c = small.tile([128, JC1], f32, name="s2_bn2")
                nc.vector.tensor_copy(s1c[:, :8], glA[:, :8])
                nc.vector.tensor_copy(s1c[:, 8:], glB[:, :8])
                nc.vector.tensor_copy(s2c[:, :8], glA[:, 8:])
                nc.vector.tensor_copy(s2c[:, 8:], glB[:, 8:])
                sv2, bv2 = bn_finish(s1c, s2c, JC1, bn2g_t, bn2b_t, "bn2")
                xn2f = res.tile([128, IC2, BL], f32, tag="bigA")
                for ic in range(IC2):
                    nc.scalar.activation(xn2f[:, ic, :], z1T[:, ic, :], AF.Relu,
                                         bias=bv2[:, ic:ic + 1], scale=sv2[:, ic:ic + 1])

                g2bc = res.tile([128, E, BL], f32, tag="gbc")
                gat2, gat2r = gating(xn2f, IC2, g2w_t, g2b_t, g2bc, "g2")

                z2r = res.tile([128, JC2, BL], f32, tag="bigB")
                expert_layer(xn2f, gat2r, g2bc, w2, b2_t, IC2, JC2, z2r, relu_out=True)

                # head: out[t] = sum_j z2r[j, t] * ow[j] + ob
                outsb = small.tile([128, TC], f32)
                with tc.tile_pool(name="psh", bufs=4, space="PSUM") as pshp:
                    for tcx in range(TC):
                        psh = pshp.tile([128, 1], f32, tag="psh", name=f"psh_{tcx}")
                        for jc in range(JC2):
                            nc.tensor.matmul(
                                psh[:], lhsT=z2r[:, jc, tcx * 128:(tcx + 1) * 128],
                                rhs=ow_t[:, jc:jc + 1],
                                start=(jc == 0), stop=(jc == JC2 - 1))
                        nc.vector.tensor_scalar(outsb[:, tcx:tcx + 1], psh[:],
                                                ob_t[:], None, OP.add)
                nc.sync.dma_start(out=out.rearrange("(c p) m -> p (c m)", p=128),
                                  in_=outsb[:])

            if py_unroll:
                for _ in range(reps):
                    emit_forward()
            elif reps > 1:
                with tc.For_i(0, reps, 1):
                    emit_forward()
            else:
                emit_forward()

    nc.finalize()
    return nc


def _get_nc(reps=1, py_unroll=False):
    key = ("nc", reps, py_unroll)
    if key not in _CACHE:
        _CACHE[key] = _build(reps, py_unroll)
    return _CACHE[key]


def kernel(x, bn1_gamma, bn1_beta, bn2_gamma, bn2_beta,
           gate1_W, gate1_b, exp1_W, exp1_b,
           gate2_W, gate2_b, exp2_W, exp2_b,
           out_W, out_b):
    from concourse.bass_utils import run_bass_kernel_spmd

    nc = _get_nc()

    xT = np.ascontiguousarray(np.asarray(x, np.float32).T)           # [DIN, B]
    w1h = _round_fp32r(np.asarray(exp1_W, np.float32).reshape(E, IC1, 128, DHID))
    w2h = _round_fp32r(np.asarray(exp2_W, np.float32).reshape(E, IC2, 128, DH2))
    b1h = np.ascontiguousarray(
        np.asarray(exp1_b, np.float32).reshape(E, JC1, 128).transpose(1, 0, 2))
    b2h = np.ascontiguousarray(
        np.asarray(exp2_b, np.float32).reshape(E, JC2, 128).transpose(1, 0, 2))
    common = {
        "xfull": xT,
        "w1": w1h, "w2": w2h, "b1": b1h, "b2": b2h,
        "g1w": np.asarray(gate1_W, np.float32).reshape(IC1, 128, E),
        "g2w": np.asarray(gate2_W, np.float32).reshape(IC2, 128, E),
        "g1b": np.asarray(gate1_b, np.float32).reshape(E, 1),
        "g2b": np.asarray(gate2_b, np.float32).reshape(E, 1),
        "bn1g": np.asarray(bn1_gamma, np.float32).reshape(IC1, 128),
        "bn1b": np.asarray(bn1_beta, np.float32).reshape(IC1, 128),
        "bn2g": np.asarray(bn2_gamma, np.float32).reshape(IC2, 128),
        "bn2b": np.asarray(bn2_beta, np.float32).reshape(IC2, 128),
        "ow": np.asarray(out_W, np.float32).reshape(JC2, 128),
        "ob": np.asarray(out_b, np.float32).reshape(1, 1),
    }
    in_maps = []
    for c in range(NCORES):
        m = dict(common)
        m["xt"] = np.ascontiguousarray(xT[:, c * BL:(c + 1) * BL])
        in_maps.append(m)

    trace = bool(int(os.environ.get("KERNEL_TRACE", "0")))
    res = run_bass_kernel_spmd(nc, in_maps, list(range(NCORES)), trace=trace)
    kernel._last = res
    return np.concatenate([res.results[c]["out"] for c in range(NCORES)], axis=0)



# revision 4
# speedup vs baseline: 1.1217x; 1.1217x over previous
"""MoE network TRN2 kernel: 8-way data-parallel over the batch.

Per core: 512 tokens. All activations kept in transposed [feature, token]
layout so BatchNorm reduces along the free dim. Expert matmuls run in
float32r (full PE rate); gating logits in float32 (exact top-2 routing).
BatchNorm statistics are the only cross-core communication (tiny AllReduce).
"""
import os
import sys

import numpy as np

sys.path.insert(0, "/opt/trn_rl_repo")

B, DIN, DHID, DH2, E = 4096, 1024, 2048, 1024, 8
NCORES = 8
BL = B // NCORES            # 512 tokens per core
IC1 = DIN // 128            # 8  input chunks, layer 1
JC1 = DHID // 128           # 16 output chunks, layer 1
IC2 = DHID // 128           # 16
JC2 = DH2 // 128            # 8
TC = BL // 128              # 4  token chunks per core
EPS = 1e-5

_CACHE = {}


def _round_fp32r(x):
    """fp32r = fp32 rounded to 11 mantissa bits, round-to-nearest-even
    (verified bit-exact against the DVE fp32->fp32r cast on hardware)."""
    b = np.ascontiguousarray(x, np.float32).view(np.uint32).astype(np.uint64)
    half = np.uint64(1 << 11)
    one = np.uint64(1)
    lsb = (b >> np.uint64(12)) & one
    b = (b + half - one + lsb) & ~np.uint64((1 << 12) - 1)
    return (b & np.uint64(0xFFFFFFFF)).astype(np.uint32).view(np.float32)


def _build(reps=1, py_unroll=False):
    import concourse.bass_isa as bass_isa
    import concourse.mybir as mybir
    import concourse.tile as tile
    from concourse import bacc
    from contextlib import nullcontext

    f32 = mybir.dt.float32
    f32r = mybir.dt.float32r
    AF = mybir.ActivationFunctionType
    OP = mybir.AluOpType
    RG = [list(range(NCORES))]

    nc = bacc.Bacc(None, target_bir_lowering=False, num_devices=NCORES)

    xt = nc.dram_tensor("xt", [DIN, BL], f32, kind="ExternalInput")
    xfull = nc.dram_tensor("xfull", [DIN, B], f32, kind="ExternalInput")
    w1 = nc.dram_tensor("w1", [E, IC1, 128, DHID], f32r, kind="ExternalInput")
    w2 = nc.dram_tensor("w2", [E, IC2, 128, DH2], f32r, kind="ExternalInput")
    b1 = nc.dram_tensor("b1", [JC1, E, 128], f32, kind="ExternalInput")
    b2 = nc.dram_tensor("b2", [JC2, E, 128], f32, kind="ExternalInput")
    g1w = nc.dram_tensor("g1w", [IC1, 128, E], f32, kind="ExternalInput")
    g2w = nc.dram_tensor("g2w", [IC2, 128, E], f32, kind="ExternalInput")
    g1b = nc.dram_tensor("g1b", [E, 1], f32, kind="ExternalInput")
    g2b = nc.dram_tensor("g2b", [E, 1], f32, kind="ExternalInput")
    bn1g = nc.dram_tensor("bn1g", [IC1, 128], f32, kind="ExternalInput")
    bn1b = nc.dram_tensor("bn1b", [IC1, 128], f32, kind="ExternalInput")
    bn2g = nc.dram_tensor("bn2g", [IC2, 128], f32, kind="ExternalInput")
    bn2b = nc.dram_tensor("bn2b", [IC2, 128], f32, kind="ExternalInput")
    ow = nc.dram_tensor("ow", [JC2, 128], f32, kind="ExternalInput")
    ob = nc.dram_tensor("ob", [1, 1], f32, kind="ExternalInput")
    out = nc.dram_tensor("out", [BL, 1], f32, kind="ExternalOutput")

    with tile.TileContext(nc) as tc:
        with tc.tile_pool(name="const", bufs=1) as const, \
             tc.tile_pool(name="res", bufs=1) as res, \
             tc.tile_pool(name="wpool", bufs=11) as wpool, \
             tc.tile_pool(name="hpool", bufs=4) as hpool, \
             tc.tile_pool(name="small", bufs=1) as small, \
             tc.tile_pool(name="gsc", bufs=10) as gsc, \
             tc.tile_pool(name="dram", bufs=1, space="DRAM") as dram:

            # ------- small parameter loads
            bn1g_t = const.tile([128, IC1], f32)
            bn1b_t = const.tile([128, IC1], f32)
            bn2g_t = const.tile([128, IC2], f32)
            bn2b_t = const.tile([128, IC2], f32)
            nc.sync.dma_start(out=bn1g_t[:], in_=bn1g.rearrange("c p -> p c"))
            nc.sync.dma_start(out=bn1b_t[:], in_=bn1b.rearrange("c p -> p c"))
            nc.sync.dma_start(out=bn2g_t[:], in_=bn2g.rearrange("c p -> p c"))
            nc.sync.dma_start(out=bn2b_t[:], in_=bn2b.rearrange("c p -> p c"))
            g1w_t = const.tile([128, IC1, E], f32)
            g2w_t = const.tile([128, IC2, E], f32)
            nc.sync.dma_start(out=g1w_t[:], in_=g1w.rearrange("c p e -> p c e"))
            nc.sync.dma_start(out=g2w_t[:], in_=g2w.rearrange("c p e -> p c e"))
            g1b_t = const.tile([E, 1], f32)
            g2b_t = const.tile([E, 1], f32)
            nc.sync.dma_start(out=g1b_t[:], in_=g1b[:])
            nc.sync.dma_start(out=g2b_t[:], in_=g2b[:])
            b1_tf = small.tile([E, JC1, 128], f32, tag="btmp", name="b1_tf")
            nc.sync.dma_start(out=b1_tf[:], in_=b1.rearrange("j e p -> e j p"))
            b1_t = const.tile([E, JC1, 128], f32r)
            nc.vector.tensor_copy(b1_t[:], b1_tf[:])
            b2_tf = small.tile([E, JC2, 128], f32, tag="btmp", name="b2_tf")
            nc.sync.dma_start(out=b2_tf[:], in_=b2.rearrange("j e p -> e j p"))
            b2_t = const.tile([E, JC2, 128], f32r)
            nc.vector.tensor_copy(b2_t[:], b2_tf[:])
            ow_t = const.tile([128, JC2], f32)
            nc.sync.dma_start(out=ow_t[:], in_=ow.rearrange("c p -> p c"))
            ob_t = const.tile([128, 1], f32)
            nc.sync.dma_start(out=ob_t[:], in_=ob[0:1, 0:1].partition_broadcast(128).squeeze(1))

            junk = res.tile([128, 1024], f32)

            # ------- batchnorm helpers
            def bn_finish(s1, s2, icn, gamma_t, beta_t, name):
                mu = small.tile([128, icn], f32, name=f"mu_{name}")
                ex2 = small.tile([128, icn], f32, name=f"ex2_{name}")
                nc.vector.tensor_scalar(mu[:], s1[:], 1.0 / B, None, OP.mult)
                nc.vector.tensor_scalar(ex2[:], s2[:], 1.0 / B, None, OP.mult)
                var = small.tile([128, icn], f32, name=f"var_{name}")
                nc.vector.tensor_tensor(out=var[:], in0=mu[:], in1=mu[:], op=OP.mult)
                nc.vector.tensor_tensor(out=var[:], in0=ex2[:], in1=var[:], op=OP.subtract)
                vare = small.tile([128, icn], f32, name=f"vare_{name}")
                nc.vector.tensor_scalar(vare[:], var[:], EPS, None, OP.add)
                sd = small.tile([128, icn], f32, name=f"sd_{name}")
                nc.scalar.activation(sd[:], vare[:], AF.Sqrt)
                rstd = small.tile([128, icn], f32, name=f"rstd_{name}")
                nc.vector.reciprocal(rstd[:], sd[:])
                sv = small.tile([128, icn], f32, name=f"sv_{name}")
                bv = small.tile([128, icn], f32, name=f"bv_{name}")
                nc.vector.tensor_tensor(out=sv[:], in0=rstd[:], in1=gamma_t[:], op=OP.mult)
                nc.vector.tensor_tensor(out=bv[:], in0=mu[:], in1=sv[:], op=OP.mult)
                nc.vector.tensor_tensor(out=bv[:], in0=beta_t[:], in1=bv[:], op=OP.subtract)
                return sv, bv

            # BN1: replicated global stats from the full (all-token) x
            def bn1_stats_replicated():
                TB = 1024
                ntb = B // TB
                s1r = small.tile([128, IC1, ntb], f32, name="s1r_bn1")
                s2r = small.tile([128, IC1, ntb], f32, name="s2r_bn1")
                for ic in range(IC1):
                    for tb in range(ntb):
                        xs = hpool.tile([128, TB], f32, tag="xstat", bufs=2,
                                        name=f"xs_{ic}_{tb}")
                        nc.scalar.dma_start(
                            out=xs[:],
                            in_=xfull[ic * 128:(ic + 1) * 128,
                                      tb * TB:(tb + 1) * TB])
                        nc.vector.tensor_reduce(
                            s1r[:, ic, tb:tb + 1], xs[:],
                            mybir.AxisListType.X, OP.add)
                        nc.scalar.activation(
                            junk[:], xs[:], AF.Square,
                            accum_out=s2r[:, ic, tb:tb + 1])
                s1 = small.tile([128, IC1], f32, name="s1_bn1")
                s2 = small.tile([128, IC1], f32, name="s2_bn1")
                nc.vector.tensor_reduce(s1[:], s1r[:], mybir.AxisListType.X, OP.add)
                nc.vector.tensor_reduce(s2[:], s2r[:], mybir.AxisListType.X, OP.add)
                return bn_finish(s1, s2, IC1, bn1g_t, bn1b_t, "bn1")

            # BN2: per-half partials, each AllReduced as soon as available
            def bn2_partial(src, jcs, name):
                icn = len(jcs)
                s1 = small.tile([128, icn], f32, name=f"s1_{name}")
                s2 = small.tile([128, icn], f32, name=f"s2_{name}")
                for k, jc in enumerate(jcs):
                    nc.vector.tensor_reduce(
                        s1[:, k:k + 1], src[:, jc, :], mybir.AxisListType.X, OP.add)
                    nc.scalar.activation(
                        junk[:, :BL], src[:, jc, :], AF.Square,
                        accum_out=s2[:, k:k + 1])
                pk = small.tile([128, 2 * icn], f32, name=f"pk_{name}")
                nc.vector.tensor_copy(pk[:, :icn], s1[:])
                nc.vector.tensor_copy(pk[:, icn:], s2[:])
                pl = dram.tile([128, 2 * icn], f32, name=f"bnp_{name}")
                ps = dram.tile([128, 2 * icn], f32, addr_space="Shared",
                               name=f"bns_{name}")
                nc.sync.dma_start(out=pl[:], in_=pk[:])
                if (reps == 1 or py_unroll) and not os.environ.get("KERNEL_NOCC"):
                    nc.gpsimd.collective_compute(
                        "AllReduce", OP.add, replica_groups=RG,
                        ins=[pl[:]], outs=[ps[:]])
                else:  # collectives desync inside For_i; timing-only stub
                    nc.sync.dma_start(out=ps[:], in_=pl[:])
                gl = small.tile([128, 2 * icn], f32, name=f"gl_{name}")
                nc.sync.dma_start(out=gl[:], in_=ps[:])
                return gl

            # ------- gating helper: logitsT [E, BL] -> top-2 masked softmax -> bcast
            def gating(xn, icn, gwt, gbt, gbc, name):
                with tc.tile_pool(name=f"psg_{name}", bufs=1, space="PSUM") as psgp:
                    psg = psgp.tile([E, BL], f32)
                    for ic in range(icn):
                        nc.tensor.matmul(psg[:], lhsT=gwt[:, ic, :], rhs=xn[:, ic, :],
                                         start=(ic == 0), stop=(ic == icn - 1))
                    lg = gsc.tile([E, BL], f32, tag="g", name=f"lg_{name}")
                    nc.vector.tensor_scalar(lg[:], psg[:], gbt[:], None, OP.add)
                m1 = gsc.tile([E, BL], f32, tag="g", name=f"m1_{name}")
                nc.gpsimd.partition_all_reduce(m1[:], lg[:], channels=E,
                                               reduce_op=bass_isa.ReduceOp.max)
                ismax = gsc.tile([E, BL], f32, tag="g", name=f"ismax_{name}")
                nc.vector.tensor_tensor(out=ismax[:], in0=lg[:], in1=m1[:], op=OP.is_equal)
                cnt = gsc.tile([E, BL], f32, tag="g", name=f"cnt_{name}")
                nc.gpsimd.partition_all_reduce(cnt[:], ismax[:], channels=E,
                                               reduce_op=bass_isa.ReduceOp.add)
                tmp = gsc.tile([E, BL], f32, tag="g", name=f"tmp_{name}")
                nc.vector.scalar_tensor_tensor(
                    out=tmp[:], in0=ismax[:], scalar=-1e30, in1=lg[:],
                    op0=OP.mult, op1=OP.add)
                m2 = gsc.tile([E, BL], f32, tag="g", name=f"m2_{name}")
                nc.gpsimd.partition_all_reduce(m2[:], tmp[:], channels=E,
                                               reduce_op=bass_isa.ReduceOp.max)
                c2m = gsc.tile([E, BL], f32, tag="g", name=f"c2m_{name}")
                nc.vector.tensor_scalar(c2m[:], cnt[:], 1.5, None, OP.is_ge)
                dif = gsc.tile([E, BL], f32, tag="g", name=f"dif_{name}")
                nc.vector.tensor_tensor(out=dif[:], in0=m1[:], in1=m2[:], op=OP.subtract)
                nc.vector.tensor_tensor(out=dif[:], in0=dif[:], in1=c2m[:], op=OP.mult)
                v2 = gsc.tile([E, BL], f32, tag="g", name=f"v2_{name}")
                nc.vector.tensor_tensor(out=v2[:], in0=dif[:], in1=m2[:], op=OP.add)
                msk = gsc.tile([E, BL], f32, tag="g", name=f"msk_{name}")
                nc.vector.tensor_tensor(out=msk[:], in0=lg[:], in1=v2[:], op=OP.is_ge)
                d = gsc.tile([E, BL], f32, tag="g", name=f"d_{name}")
                nc.vector.tensor_tensor(out=d[:], in0=lg[:], in1=m1[:], op=OP.subtract)
                exd = gsc.tile([E, BL], f32, tag="g", name=f"exd_{name}")
                nc.scalar.activation(exd[:], d[:], AF.Exp)
                exm = gsc.tile([E, BL], f32, tag="g", name=f"exm_{name}")
                nc.vector.tensor_tensor(out=exm[:], in0=exd[:], in1=msk[:], op=OP.mult)
                den = gsc.tile([E, BL], f32, tag="g", name=f"den_{name}")
                nc.gpsimd.partition_all_reduce(den[:], exm[:], channels=E,
                                               reduce_op=bass_isa.ReduceOp.add)
                rden = gsc.tile([E, BL], f32, tag="g", name=f"rden_{name}")
                nc.vector.reciprocal(rden[:], den[:])
                gat = gsc.tile([E, BL], f32, tag="g", name=f"gat_{name}")
                nc.vector.tensor_tensor(out=gat[:], in0=exm[:], in1=rden[:], op=OP.mult)
                gatr = small.tile([E, BL], f32r, name=f"gatr_{name}")
                nc.vector.tensor_copy(gatr[:], gat[:])
                gd = dram.tile([E, BL], f32, name=f"gd_{name}")
                nc.sync.dma_start(out=gd[:], in_=gat[:])
                for e in range(E):
                    nc.sync.dma_start(
                        out=gbc[:, e, :],
                        in_=gd[e:e + 1, :].partition_broadcast(128).squeeze(1))
                return gat, gatr

            # ------- expert layer helper
            def expert_layer(xn, gat, gbc, wdram, bt, icn, jcn, zdst, relu_out,
                             jh_cb=None):
                n_jh = (jcn + 7) // 8
                with tc.tile_pool(name=f"psm_{len(zdst.shape)}_{icn}", bufs=8,
                                  space="PSUM") as psp:
                    for jh in range(n_jh):
                        njc = min(8, jcn - jh * 8)
                        pss = [psp.tile([128, BL], f32, tag="ps",
                                        name=f"ps_{jh}_{j}") for j in range(njc)]
                        for jc in range(njc):
                            nc.tensor.matmul(pss[jc][:], lhsT=bt[:, jh * 8 + jc, :],
                                             rhs=gat[:], start=True, stop=False)
                        for e in range(E):
                            for ic in range(icn):
                                ws = wpool.tile([128, njc * 128], f32r, tag="ws",
                                                name=f"ws_{jh}_{e}_{ic}")
                                nc.sync.dma_start(
                                    out=ws[:],
                                    in_=wdram[e, ic, :, jh * 1024:jh * 1024 + njc * 128])
                                ht = hpool.tile([128, BL], f32r, tag="ht",
                                                name=f"ht_{jh}_{e}_{ic}")
                                nc.vector.tensor_tensor(
                                    out=ht[:], in0=xn[:, ic, :], in1=gbc[:, e, :],
                                    op=OP.mult)
                                last = (e == E - 1 and ic == icn - 1)
                                for jc in range(njc):
                                    nc.tensor.matmul(
                                        pss[jc][:],
                                        lhsT=ws[:, jc * 128:(jc + 1) * 128],
                                        rhs=ht[:], start=False, stop=last)
                        for jc in range(njc):
                            if relu_out:
                                nc.scalar.activation(zdst[:, jh * 8 + jc, :],
                                                     pss[jc][:], AF.Relu)
                            else:
                                nc.vector.tensor_copy(zdst[:, jh * 8 + jc, :],
                                                      pss[jc][:])
                        if jh_cb is not None:
                            jh_cb(jh, [jh * 8 + j for j in range(njc)])

            def emit_forward():
                # =================== forward pass ===================
                # x load + BN1 stats
                xtf = res.tile([128, IC1, BL], f32, tag="bigA")
                for ic in range(IC1):
                    nc.scalar.dma_start(out=xtf[:, ic, :],
                                        in_=xt[ic * 128:(ic + 1) * 128, :])
                sv1, bv1 = bn1_stats_replicated()

                # normalize (fp32, exact for gating)
                xnf = res.tile([128, IC1, BL], f32, tag="bigB")
                for ic in range(IC1):
                    nc.vector.tensor_scalar(xnf[:, ic, :], xtf[:, ic, :],
                                            sv1[:, ic:ic + 1], bv1[:, ic:ic + 1],
                                            OP.mult, OP.add)

                g1bc = res.tile([128, E, BL], f32, tag="gbc")
                gat1, gat1r = gating(xnf, IC1, g1w_t, g1b_t, g1bc, "g1")

                z1T = res.tile([128, JC1, BL], f32)
                bn2_gls = {}

                def bn2_cb(jh, jcs):
                    bn2_gls[jh] = bn2_partial(z1T, jcs, f"bn2h{jh}")

                expert_layer(xnf, gat1r, g1bc, w1, b1_t, IC1, JC1, z1T,
                             relu_out=False, jh_cb=bn2_cb)

                # BN2 + ReLU: combine the two halves' global partials
                glA, glB = bn2_gls[0], bn2_gls[1]
                s1c = small.tile([128, JC1], f32, name="s1_bn2")
                s2c = small.tile([128, JC1], f32, name="s2_bn2")
                nc.vector.tensor_copy(s1c[:, :8], glA[:, :8])
                nc.vector.tensor_copy(s1c[:, 8:], glB[:, :8])
                nc.vector.tensor_copy(s2c[:, :8], glA[:, 8:])
                nc.vector.tensor_copy(s2c[:, 8:], glB[:, 8:])
                sv2, bv2 = bn_finish(s1c, s2c, JC1, bn2g_t, bn2b_t, "bn2")
                xn2f = res.tile([128, IC2, BL], f32, tag="bigA")
                for ic in range(IC2):
                    nc.scalar.activation(xn2f[:, ic, :], z1T[:, ic, :], AF.Relu,
                                         bias=bv2[:, ic:ic + 1], scale=sv2[:, ic:ic + 1])

                g2bc = res.tile([128, E, BL], f32, tag="gbc")
                gat2, gat2r = gating(xn2f, IC2, g2w_t, g2b_t, g2bc, "g2")

                z2r = res.tile([128, JC2, BL], f32, tag="bigB")
                expert_layer(xn2f, gat2r, g2bc, w2, b2_t, IC2, JC2, z2r, relu_out=True)

                # head: out[t] = sum_j z2r[j, t] * ow[j] + ob
                outsb = small.tile([128, TC], f32)
                with tc.tile_pool(name="psh", bufs=4, space="PSUM") as pshp:
                    for tcx in range(TC):
                        psh = pshp.tile([128, 1], f32, tag="psh", name=f"psh_{tcx}")
                        for jc in range(JC2):
                            nc.tensor.matmul(
                                psh[:], lhsT=z2r[:, jc, tcx * 128:(tcx + 1) * 128],
                                rhs=ow_t[:, jc:jc + 1],
                                start=(jc == 0), stop=(jc == JC2 - 1))
                        nc.vector.tensor_scalar(outsb[:, tcx:tcx + 1], psh[:],
                                                ob_t[:], None, OP.add)
                nc.sync.dma_start(out=out.rearrange("(c p) m -> p (c m)", p=128),
                                  in_=outsb[:])

            if py_unroll:
                for _ in range(reps):
                    emit_forward()
            elif reps > 1:
                with tc.For_i(0, reps, 1):
                    emit_forward()
            else:
                emit_forward()

    nc.finalize()
    return nc


def _get_nc(reps=1, py_unroll=False):
    key = ("nc", reps, py_unroll)
    if key not in _CACHE:
        _CACHE[key] = _build(reps, py_unroll)
    return _CACHE[key]


def kernel(x, bn1_gamma, bn1_beta, bn2_gamma, bn2_beta,
           gate1_W, gate1_b, exp1_W, exp1_b,
           gate2_W, gate2_b, exp2_W, exp2_b,
           out_W, out_b):
    from concourse.bass_utils import run_bass_kernel_spmd

    nc = _get_nc()

    xT = np.ascontiguousarray(np.asarray(x, np.float32).T)           # [DIN, B]
    w1h = _round_fp32r(np.asarray(exp1_W, np.float32).reshape(E, IC1, 128, DHID))
    w2h = _round_fp32r(np.asarray(exp2_W, np.float32).reshape(E, IC2, 128, DH2))
    b1h = np.ascontiguousarray(
        np.asarray(exp1_b, np.float32).reshape(E, JC1, 128).transpose(1, 0, 2))
    b2h = np.ascontiguousarray(
        np.asarray(exp2_b, np.float32).reshape(E, JC2, 128).transpose(1, 0, 2))
    common = {
        "xfull": xT,
        "w1": w1h, "w2": w2h, "b1": b1h, "b2": b2h,
        "g1w": np.asarray(gate1_W, np.float32).reshape(IC1, 128, E),
        "g2w": np.asarray(gate2_W, np.float32).reshape(IC2, 128, E),
        "g1b": np.asarray(gate1_b, np.float32).reshape(E, 1),
        "g2b": np.asarray(gate2_b, np.float32).reshape(E, 1),
        "bn1g": np.asarray(bn1_gamma, np.float32).reshape(IC1, 128),
        "bn1b": np.asarray(bn1_beta, np.float32).reshape(IC1, 128),
        "bn2g": np.asarray(bn2_gamma, np.float32).reshape(IC2, 128),
        "bn2b": np.asarray(bn2_beta, np.float32).reshape(IC2, 128),
        "ow": np.asarray(out_W, np.float32).reshape(JC2, 128),
        "ob": np.asarray(out_b, np.float32).reshape(1, 1),
    }
    in_maps = []
    for c in range(NCORES):
        m = dict(common)
        m["xt"] = np.ascontiguousarray(xT[:, c * BL:(c + 1) * BL])
        in_maps.append(m)

    trace = bool(int(os.environ.get("KERNEL_TRACE", "0")))
    res = run_bass_kernel_spmd(nc, in_maps, list(range(NCORES)), trace=trace)
    kernel._last = res
    return np.concatenate([res.results[c]["out"] for c in range(NCORES)], axis=0)



# revision 6
# speedup vs baseline: 1.1733x; 1.0460x over previous
"""MoE network TRN2 kernel: 8-way data-parallel over the batch.

Per core: 512 tokens. All activations kept in transposed [feature, token]
layout so BatchNorm reduces along the free dim. Expert matmuls run in
float32r (full PE rate); gating logits in float32 (exact top-2 routing).
BatchNorm statistics are the only cross-core communication (tiny AllReduce).
"""
import os
import sys

import numpy as np

sys.path.insert(0, "/opt/trn_rl_repo")

B, DIN, DHID, DH2, E = 4096, 1024, 2048, 1024, 8
NCORES = 8
BL = B // NCORES            # 512 tokens per core
IC1 = DIN // 128            # 8  input chunks, layer 1
JC1 = DHID // 128           # 16 output chunks, layer 1
IC2 = DHID // 128           # 16
JC2 = DH2 // 128            # 8
TC = BL // 128              # 4  token chunks per core
EPS = 1e-5

_CACHE = {}


def _round_fp32r(x):
    """fp32r = fp32 rounded to 11 mantissa bits, round-to-nearest-even
    (verified bit-exact against the DVE fp32->fp32r cast on hardware)."""
    b = np.ascontiguousarray(x, np.float32).view(np.uint32).astype(np.uint64)
    half = np.uint64(1 << 11)
    one = np.uint64(1)
    lsb = (b >> np.uint64(12)) & one
    b = (b + half - one + lsb) & ~np.uint64((1 << 12) - 1)
    return (b & np.uint64(0xFFFFFFFF)).astype(np.uint32).view(np.float32)


def _build(reps=1, py_unroll=False):
    import concourse.bass_isa as bass_isa
    import concourse.mybir as mybir
    import concourse.tile as tile
    from concourse import bacc
    from contextlib import nullcontext

    f32 = mybir.dt.float32
    f32r = mybir.dt.float32r
    AF = mybir.ActivationFunctionType
    OP = mybir.AluOpType
    RG = [list(range(NCORES))]

    nc = bacc.Bacc(None, target_bir_lowering=False, num_devices=NCORES)

    xt = nc.dram_tensor("xt", [DIN, BL], f32, kind="ExternalInput")
    xfull = nc.dram_tensor("xfull", [DIN, B], f32, kind="ExternalInput")
    w1 = nc.dram_tensor("w1", [E, IC1, 128, DHID], f32r, kind="ExternalInput")
    w2 = nc.dram_tensor("w2", [E, IC2, 128, DH2], f32r, kind="ExternalInput")
    b1 = nc.dram_tensor("b1", [JC1, E, 128], f32, kind="ExternalInput")
    b2 = nc.dram_tensor("b2", [JC2, E, 128], f32, kind="ExternalInput")
    g1w = nc.dram_tensor("g1w", [IC1, 128, E], f32, kind="ExternalInput")
    g2w = nc.dram_tensor("g2w", [IC2, 128, E], f32, kind="ExternalInput")
    g1b = nc.dram_tensor("g1b", [E, 1], f32, kind="ExternalInput")
    g2b = nc.dram_tensor("g2b", [E, 1], f32, kind="ExternalInput")
    bn1g = nc.dram_tensor("bn1g", [IC1, 128], f32, kind="ExternalInput")
    bn1b = nc.dram_tensor("bn1b", [IC1, 128], f32, kind="ExternalInput")
    bn2g = nc.dram_tensor("bn2g", [IC2, 128], f32, kind="ExternalInput")
    bn2b = nc.dram_tensor("bn2b", [IC2, 128], f32, kind="ExternalInput")
    ow = nc.dram_tensor("ow", [JC2, 128], f32, kind="ExternalInput")
    ob = nc.dram_tensor("ob", [1, 1], f32, kind="ExternalInput")
    out = nc.dram_tensor("out", [BL, 1], f32, kind="ExternalOutput")

    with tile.TileContext(nc) as tc:
        with tc.tile_pool(name="const", bufs=1) as const, \
             tc.tile_pool(name="res", bufs=1) as res, \
             tc.tile_pool(name="wpool", bufs=11) as wpool, \
             tc.tile_pool(name="hpool", bufs=4) as hpool, \
             tc.tile_pool(name="small", bufs=1) as small, \
             tc.tile_pool(name="gsc", bufs=10) as gsc, \
             tc.tile_pool(name="dram", bufs=1, space="DRAM") as dram:

            # ------- small parameter loads
            bn1g_t = const.tile([128, IC1], f32)
            bn1b_t = const.tile([128, IC1], f32)
            bn2g_t = const.tile([128, IC2], f32)
            bn2b_t = const.tile([128, IC2], f32)
            nc.sync.dma_start(out=bn1g_t[:], in_=bn1g.rearrange("c p -> p c"))
            nc.sync.dma_start(out=bn1b_t[:], in_=bn1b.rearrange("c p -> p c"))
            nc.sync.dma_start(out=bn2g_t[:], in_=bn2g.rearrange("c p -> p c"))
            nc.sync.dma_start(out=bn2b_t[:], in_=bn2b.rearrange("c p -> p c"))
            g1w_t = const.tile([128, IC1, E], f32)
            g2w_t = const.tile([128, IC2, E], f32)
            nc.sync.dma_start(out=g1w_t[:], in_=g1w.rearrange("c p e -> p c e"))
            nc.sync.dma_start(out=g2w_t[:], in_=g2w.rearrange("c p e -> p c e"))
            g1b_t = const.tile([E, 1], f32)
            g2b_t = const.tile([E, 1], f32)
            nc.sync.dma_start(out=g1b_t[:], in_=g1b[:])
            nc.sync.dma_start(out=g2b_t[:], in_=g2b[:])
            b1_tf = small.tile([E, JC1, 128], f32, tag="btmp", name="b1_tf")
            nc.sync.dma_start(out=b1_tf[:], in_=b1.rearrange("j e p -> e j p"))
            b1_t = const.tile([E, JC1, 128], f32r)
            nc.vector.tensor_copy(b1_t[:], b1_tf[:])
            b2_tf = small.tile([E, JC2, 128], f32, tag="btmp", name="b2_tf")
            nc.sync.dma_start(out=b2_tf[:], in_=b2.rearrange("j e p -> e j p"))
            b2_t = const.tile([E, JC2, 128], f32r)
            nc.vector.tensor_copy(b2_t[:], b2_tf[:])
            ow_t = const.tile([128, JC2], f32)
            nc.sync.dma_start(out=ow_t[:], in_=ow.rearrange("c p -> p c"))
            ob_t = const.tile([128, 1], f32)
            nc.sync.dma_start(out=ob_t[:], in_=ob[0:1, 0:1].partition_broadcast(128).squeeze(1))

            junk = res.tile([128, 1024], f32)

            # ------- batchnorm helpers
            def bn_finish(s1, s2, icn, gamma_t, beta_t, name):
                mu = small.tile([128, icn], f32, name=f"mu_{name}")
                ex2 = small.tile([128, icn], f32, name=f"ex2_{name}")
                nc.vector.tensor_scalar(mu[:], s1[:], 1.0 / B, None, OP.mult)
                nc.vector.tensor_scalar(ex2[:], s2[:], 1.0 / B, None, OP.mult)
                var = small.tile([128, icn], f32, name=f"var_{name}")
                nc.vector.tensor_tensor(out=var[:], in0=mu[:], in1=mu[:], op=OP.mult)
                nc.vector.tensor_tensor(out=var[:], in0=ex2[:], in1=var[:], op=OP.subtract)
                vare = small.tile([128, icn], f32, name=f"vare_{name}")
                nc.vector.tensor_scalar(vare[:], var[:], EPS, None, OP.add)
                sd = small.tile([128, icn], f32, name=f"sd_{name}")
                nc.scalar.activation(sd[:], vare[:], AF.Sqrt)
                rstd = small.tile([128, icn], f32, name=f"rstd_{name}")
                nc.vector.reciprocal(rstd[:], sd[:])
                sv = small.tile([128, icn], f32, name=f"sv_{name}")
                bv = small.tile([128, icn], f32, name=f"bv_{name}")
                nc.vector.tensor_tensor(out=sv[:], in0=rstd[:], in1=gamma_t[:], op=OP.mult)
                nc.vector.tensor_tensor(out=bv[:], in0=mu[:], in1=sv[:], op=OP.mult)
                nc.vector.tensor_tensor(out=bv[:], in0=beta_t[:], in1=bv[:], op=OP.subtract)
                return sv, bv

            # BN1: replicated global stats from the full (all-token) x
            def bn1_stats_replicated():
                TB = 1024
                ntb = B // TB
                s1r = small.tile([128, IC1, ntb], f32, name="s1r_bn1")
                s2r = small.tile([128, IC1, ntb], f32, name="s2r_bn1")
                for ic in range(IC1):
                    for tb in range(ntb):
                        xs = hpool.tile([128, TB], f32, tag="xstat", bufs=2,
                                        name=f"xs_{ic}_{tb}")
                        nc.scalar.dma_start(
                            out=xs[:],
                            in_=xfull[ic * 128:(ic + 1) * 128,
                                      tb * TB:(tb + 1) * TB])
                        nc.vector.tensor_reduce(
                            s1r[:, ic, tb:tb + 1], xs[:],
                            mybir.AxisListType.X, OP.add)
                        nc.scalar.activation(
                            junk[:], xs[:], AF.Square,
                            accum_out=s2r[:, ic, tb:tb + 1])
                s1 = small.tile([128, IC1], f32, name="s1_bn1")
                s2 = small.tile([128, IC1], f32, name="s2_bn1")
                nc.vector.tensor_reduce(s1[:], s1r[:], mybir.AxisListType.X, OP.add)
                nc.vector.tensor_reduce(s2[:], s2r[:], mybir.AxisListType.X, OP.add)
                return bn_finish(s1, s2, IC1, bn1g_t, bn1b_t, "bn1")

            # BN2: per-half partials, each AllReduced as soon as available
            def bn2_partial(src, jcs, name):
                icn = len(jcs)
                s1 = small.tile([128, icn], f32, name=f"s1_{name}")
                s2 = small.tile([128, icn], f32, name=f"s2_{name}")
                for k, jc in enumerate(jcs):
                    nc.vector.tensor_reduce(
                        s1[:, k:k + 1], src[:, jc, :], mybir.AxisListType.X, OP.add)
                    nc.scalar.activation(
                        junk[:, :BL], src[:, jc, :], AF.Square,
                        accum_out=s2[:, k:k + 1])
                pk = small.tile([128, 2 * icn], f32, name=f"pk_{name}")
                nc.vector.tensor_copy(pk[:, :icn], s1[:])
                nc.vector.tensor_copy(pk[:, icn:], s2[:])
                pl = dram.tile([128, 2 * icn], f32, name=f"bnp_{name}")
                ps = dram.tile([128, 2 * icn], f32, addr_space="Shared",
                               name=f"bns_{name}")
                nc.sync.dma_start(out=pl[:], in_=pk[:])
                if (reps == 1 or py_unroll) and not os.environ.get("KERNEL_NOCC"):
                    nc.gpsimd.collective_compute(
                        "AllReduce", OP.add, replica_groups=RG,
                        ins=[pl[:]], outs=[ps[:]])
                else:  # collectives desync inside For_i; timing-only stub
                    nc.sync.dma_start(out=ps[:], in_=pl[:])
                gl = small.tile([128, 2 * icn], f32, name=f"gl_{name}")
                nc.sync.dma_start(out=gl[:], in_=ps[:])
                return gl

            # ------- gating helper: logitsT [E, BL] -> top-2 masked softmax -> bcast
            def gating(xn, icn, gwt, gbt, gbc, name):
                with tc.tile_pool(name=f"psg_{name}", bufs=1, space="PSUM") as psgp:
                    psg = psgp.tile([E, BL], f32)
                    for ic in range(icn):
                        nc.tensor.matmul(psg[:], lhsT=gwt[:, ic, :], rhs=xn[:, ic, :],
                                         start=(ic == 0), stop=(ic == icn - 1))
                    lg = gsc.tile([E, BL], f32, tag="g", name=f"lg_{name}")
                    nc.vector.tensor_scalar(lg[:], psg[:], gbt[:], None, OP.add)
                m1 = gsc.tile([E, BL], f32, tag="g", name=f"m1_{name}")
                nc.gpsimd.partition_all_reduce(m1[:], lg[:], channels=E,
                                               reduce_op=bass_isa.ReduceOp.max)
                ismax = gsc.tile([E, BL], f32, tag="g", name=f"ismax_{name}")
                nc.vector.tensor_tensor(out=ismax[:], in0=lg[:], in1=m1[:], op=OP.is_equal)
                cnt = gsc.tile([E, BL], f32, tag="g", name=f"cnt_{name}")
                nc.gpsimd.partition_all_reduce(cnt[:], ismax[:], channels=E,
                                               reduce_op=bass_isa.ReduceOp.add)
                tmp = gsc.tile([E, BL], f32, tag="g", name=f"tmp_{name}")
                nc.vector.scalar_tensor_tensor(
                    out=tmp[:], in0=ismax[:], scalar=-1e30, in1=lg[:],
                    op0=OP.mult, op1=OP.add)
                m2 = gsc.tile([E, BL], f32, tag="g", name=f"m2_{name}")
                nc.gpsimd.partition_all_reduce(m2[:], tmp[:], channels=E,
                                               reduce_op=bass_isa.ReduceOp.max)
                c2m = gsc.tile([E, BL], f32, tag="g", name=f"c2m_{name}")
                nc.vector.tensor_scalar(c2m[:], cnt[:], 1.5, None, OP.is_ge)
                dif = gsc.tile([E, BL], f32, tag="g", name=f"dif_{name}")
                nc.vector.tensor_tensor(out=dif[:], in0=m1[:], in1=m2[:], op=OP.subtract)
                nc.vector.tensor_tensor(out=dif[:], in0=dif[:], in1=c2m[:], op=OP.mult)
                v2 = gsc.tile([E, BL], f32, tag="g", name=f"v2_{name}")
                nc.vector.tensor_tensor(out=v2[:], in0=dif[:], in1=m2[:], op=OP.add)
                msk = gsc.tile([E, BL], f32, tag="g", name=f"msk_{name}")
                nc.vector.tensor_tensor(out=msk[:], in0=lg[:], in1=v2[:], op=OP.is_ge)
                d = gsc.tile([E, BL], f32, tag="g", name=f"d_{name}")
                nc.vector.tensor_tensor(out=d[:], in0=lg[:], in1=m1[:], op=OP.subtract)
                exd = gsc.tile([E, BL], f32, tag="g", name=f"exd_{name}")
                nc.scalar.activation(exd[:], d[:], AF.Exp)
                exm = gsc.tile([E, BL], f32, tag="g", name=f"exm_{name}")
                nc.vector.tensor_tensor(out=exm[:], in0=exd[:], in1=msk[:], op=OP.mult)
                den = gsc.tile([E, BL], f32, tag="g", name=f"den_{name}")
                nc.gpsimd.partition_all_reduce(den[:], exm[:], channels=E,
                                               reduce_op=bass_isa.ReduceOp.add)
                rden = gsc.tile([E, BL], f32, tag="g", name=f"rden_{name}")
                nc.vector.reciprocal(rden[:], den[:])
                gat = gsc.tile([E, BL], f32, tag="g", name=f"gat_{name}")
                nc.vector.tensor_tensor(out=gat[:], in0=exm[:], in1=rden[:], op=OP.mult)
                gatr = small.tile([E, BL], f32r, name=f"gatr_{name}")
                nc.vector.tensor_copy(gatr[:], gat[:])
                gd = dram.tile([E, BL], f32, name=f"gd_{name}")
                nc.sync.dma_start(out=gd[:], in_=gat[:])
                for e in range(E):
                    nc.sync.dma_start(
                        out=gbc[:, e, :],
                        in_=gd[e:e + 1, :].partition_broadcast(128).squeeze(1))
                return gat, gatr

            # ------- expert layer helper
            def expert_layer(xn, gat, gbc, wdram, bt, icn, jcn, zdst, relu_out,
                             jh_cb=None):
                n_jh = (jcn + 7) // 8
                with tc.tile_pool(name=f"psm_{len(zdst.shape)}_{icn}", bufs=8,
                                  space="PSUM") as psp:
                    for jh in range(n_jh):
                        njc = min(8, jcn - jh * 8)
                        pss = [psp.tile([128, BL], f32, tag="ps",
                                        name=f"ps_{jh}_{j}") for j in range(njc)]
                        for jc in range(njc):
                            nc.tensor.matmul(pss[jc][:], lhsT=bt[:, jh * 8 + jc, :],
                                             rhs=gat[:], start=True, stop=False)
                        for e in range(E):
                            for ic in range(icn):
                                ws = wpool.tile([128, njc * 128], f32r, tag="ws",
                                                name=f"ws_{jh}_{e}_{ic}")
                                nc.sync.dma_start(
                                    out=ws[:],
                                    in_=wdram[e, ic, :, jh * 1024:jh * 1024 + njc * 128])
                                ht = hpool.tile([128, BL], f32r, tag="ht",
                                                name=f"ht_{jh}_{e}_{ic}")
                                nc.vector.tensor_tensor(
                                    out=ht[:], in0=xn[:, ic, :], in1=gbc[:, e, :],
                                    op=OP.mult)
                                last = (e == E - 1 and ic == icn - 1)
                                for jc in range(njc):
                                    nc.tensor.matmul(
                                        pss[jc][:],
                                        lhsT=ws[:, jc * 128:(jc + 1) * 128],
                                        rhs=ht[:], start=False, stop=last)
                        for jc in range(njc):
                            if relu_out:
                                nc.scalar.activation(zdst[:, jh * 8 + jc, :],
                                                     pss[jc][:], AF.Relu)
                            else:
                                nc.vector.tensor_copy(zdst[:, jh * 8 + jc, :],
                                                      pss[jc][:])
                        if jh_cb is not None:
                            jh_cb(jh, [jh * 8 + j for j in range(njc)])

            def emit_forward():
                # =================== forward pass ===================
                # x load + BN1 stats
                xtf = res.tile([128, IC1, BL], f32, tag="bigA")
                for ic in range(IC1):
                    nc.scalar.dma_start(out=xtf[:, ic, :],
                                        in_=xt[ic * 128:(ic + 1) * 128, :])
                sv1, bv1 = bn1_stats_replicated()

                # normalize (fp32, exact for gating)
                xnf = res.tile([128, IC1, BL], f32, tag="bigB")
                for ic in range(IC1):
                    nc.vector.tensor_scalar(xnf[:, ic, :], xtf[:, ic, :],
                                            sv1[:, ic:ic + 1], bv1[:, ic:ic + 1],
                                            OP.mult, OP.add)

                g1bc = res.tile([128, E, BL], f32, tag="gbc")
                gat1, gat1r = gating(xnf, IC1, g1w_t, g1b_t, g1bc, "g1")

                z1T = res.tile([128, JC1, BL], f32)
                bn2_gls = {}

                def bn2_cb(jh, jcs):
                    bn2_gls[jh] = bn2_partial(z1T, jcs, f"bn2h{jh}")

                expert_layer(xnf, gat1r, g1bc, w1, b1_t, IC1, JC1, z1T,
                             relu_out=False, jh_cb=bn2_cb)

                # BN2 + ReLU: combine the two halves' global partials
                glA, glB = bn2_gls[0], bn2_gls[1]
                s1c = small.tile([128, JC1], f32, name="s1_bn2")
                s2c = small.tile([128, JC1], f32, name="s2_bn2")
                nc.vector.tensor_copy(s1c[:, :8], glA[:, :8])
                nc.vector.tensor_copy(s1c[:, 8:], glB[:, :8])
                nc.vector.tensor_copy(s2c[:, :8], glA[:, 8:])
                nc.vector.tensor_copy(s2c[:, 8:], glB[:, 8:])
                sv2, bv2 = bn_finish(s1c, s2c, JC1, bn2g_t, bn2b_t, "bn2")
                xn2f = res.tile([128, IC2, BL], f32, tag="bigA")
                for ic in range(IC2):
                    nc.scalar.activation(xn2f[:, ic, :], z1T[:, ic, :], AF.Relu,
                                         bias=bv2[:, ic:ic + 1], scale=sv2[:, ic:ic + 1])

                g2bc = res.tile([128, E, BL], f32, tag="gbc")
                gat2, gat2r = gating(xn2f, IC2, g2w_t, g2b_t, g2bc, "g2")

                z2r = res.tile([128, JC2, BL], f32, tag="bigB")
                expert_layer(xn2f, gat2r, g2bc, w2, b2_t, IC2, JC2, z2r, relu_out=True)

                # head: out[t] = sum_j z2r[j, t] * ow[j] + ob
                outsb = small.tile([128, TC], f32)
                with tc.tile_pool(name="psh", bufs=4, space="PSUM") as pshp:
                    for tcx in range(TC):
                        psh = pshp.tile([128, 1], f32, tag="psh", name=f"psh_{tcx}")
                        for jc in range(JC2):
                            nc.tensor.matmul(
                                psh[:], lhsT=z2r[:, jc, tcx * 128:(tcx + 1) * 128],
                                rhs=ow_t[:, jc:jc + 1],
                                start=(jc == 0), stop=(jc == JC2 - 1))
                        nc.vector.tensor_scalar(outsb[:, tcx:tcx + 1], psh[:],
                                                ob_t[:], None, OP.add)
                nc.sync.dma_start(out=out.rearrange("(c p) m -> p (c m)", p=128),
                                  in_=outsb[:])

            if py_unroll:
                for _ in range(reps):
                    emit_forward()
            elif reps > 1:
                with tc.For_i(0, reps, 1):
                    emit_forward()
            else:
                emit_forward()

    nc.finalize()
    return nc


def _get_nc(reps=1, py_unroll=False):
    key = ("nc", reps, py_unroll)
    if key not in _CACHE:
        _CACHE[key] = _build(reps, py_unroll)
    return _CACHE[key]


def kernel(x, bn1_gamma, bn1_beta, bn2_gamma, bn2_beta,
           gate1_W, gate1_b, exp1_W, exp1_b,
           gate2_W, gate2_b, exp2_W, exp2_b,
           out_W, out_b):
    from concourse.bass_utils import run_bass_kernel_spmd

    nc = _get_nc()

    xT = np.ascontiguousarray(np.asarray(x, np.float32).T)           # [DIN, B]
    w1h = _round_fp32r(np.asarray(exp1_W, np.float32).reshape(E, IC1, 128, DHID))
    w2h = _round_fp32r(np.asarray(exp2_W, np.float32).reshape(E, IC2, 128, DH2))
    b1h = np.ascontiguousarray(
        np.asarray(exp1_b, np.float32).reshape(E, JC1, 128).transpose(1, 0, 2))
    b2h = np.ascontiguousarray(
        np.asarray(exp2_b, np.float32).reshape(E, JC2, 128).transpose(1, 0, 2))
    common = {
        "xfull": xT,
        "w1": w1h, "w2": w2h, "b1": b1h, "b2": b2h,
        "g1w": np.asarray(gate1_W, np.float32).reshape(IC1, 128, E),
        "g2w": np.asarray(gate2_W, np.float32).reshape(IC2, 128, E),
        "g1b": np.asarray(gate1_b, np.float32).reshape(E, 1),
        "g2b": np.asarray(gate2_b, np.float32).reshape(E, 1),
        "bn1g": np.asarray(bn1_gamma, np.float32).reshape(IC1, 128),
        "bn1b": np.asarray(bn1_beta, np.float32).reshape(IC1, 128),
        "bn2g": np.asarray(bn2_gamma, np.float32).reshape(IC2, 128),
        "bn2b": np.asarray(bn2_beta, np.float32).reshape(IC2, 128),
        "ow": np.asarray(out_W, np.float32).reshape(JC2, 128),
        "ob": np.asarray(out_b, np.float32).reshape(1, 1),
    }
    in_maps = []
    for c in range(NCORES):
        m = dict(common)
        m["xt"] = np.ascontiguousarray(xT[:, c * BL:(c + 1) * BL])
        in_maps.append(m)

    trace = bool(int(os.environ.get("KERNEL_TRACE", "0")))
    res = run_bass_kernel_spmd(nc, in_maps, list(range(NCORES)), trace=trace)
    kernel._last = res
    return np.concatenate([res.results[c]["out"] for c in range(NCORES)], axis=0)

